# revision 37
# baseline (speedup 1.0000x reference)
"""Criss-Cross Attention (B=4, C=256, H=W=96, 8 heads) on 8 trn2 NeuronCores.

Sharding: core = (batch b, head-group hg of 4 heads); 8 cores = 4 b x 2 hg.
Each core computes q/k/v projections for its 128 output channels over the
full image, then row- and column-attention for its 4 heads, then
gamma*out + x for its channel slice. Host splits inputs / concatenates
outputs; no collectives.

Key perf structure: a persistent block-diagonal q tensor (qd_full) is
built with a few large DMAs per phase (DMA *issue* costs ~640ns on the
issuing queue regardless of size, so instruction count is what matters);
x is streamed in chunks on the scalar DMA queue; the column-half
self-mask is a post-exp 0/1 multiply on gpsimd; energies run two groups
ahead of aggregation so the PE never starves.

Self-contained: includes the TileContext drain-wait splitting workaround.
"""

import numpy as np
import ml_dtypes

import concourse.bass as bass
import concourse.mybir as mybir
import concourse.tile as tile
from concourse.vector_clock import ScopedClock
from concourse.bass_utils import run_bass_kernel_spmd

BF16 = mybir.dt.bfloat16
F32 = mybir.dt.float32
AF = mybir.ActivationFunctionType
OP = mybir.AluOpType

B, C, H, W = 4, 256, 96, 96
HW = H * W  # 9216
HC = 128  # output channels per core (4 heads x 32)
NHG = 4  # heads per core
HD = 32
GRP = 2  # rows per pipeline group
NG = H // GRP  # 48 groups
PADF = HW + 32  # k free-dim pad so [.., 96y : 96y+128] reads stay in-bounds
CHUNK = 1536  # pixels per x streaming chunk (16 rows)
NCH = HW // CHUNK  # 6
CPADF = CHUNK + 32  # chunk pad for vT's 128-wide row windows
FBLK = 6  # finalize groups per xres-load / out-store block

_cached = {}


# ---------------------------------------------------------------- tile patch
def _patched_drain_and_barrier(self, tick_clock, wait_clock):
    # walrus here rejects >1 sem wait on the final drain; spread the
    # global-clock waits across single-wait nops.
    drain_inst = self.nc.sync.drain()
    wait_clock.add_sem_waits(
        drain_inst.ins, ScopedClock({None: tick_clock.global_clock})
    )
    si = drain_inst.ins.sync_info
    waits = list(si.on_wait)
    if len(waits) > 1:
        drain_inst.ins.sync_info = mybir.SyncInfo(
            on_wait=waits[:1], on_update=list(si.on_update)
        )
        for i in range(1, len(waits)):
            nop = self.nc.sync.nop(nofuse=True)
            nop.ins.sync_info = mybir.SyncInfo(on_wait=waits[i : i + 1], on_update=[])
    self.nc.all_engine_barrier()
    assert self.sems is not None
    popped = self.nc._tile_sem_poison_stack.pop()
    assert popped is self._sem_poison
    self.nc.clear_and_free_semaphores(list(self.sems.allocated().values()))
    self.nc.all_engine_barrier()


tile.TileContext._drain_and_barrier = _patched_drain_and_barrier


def _split_excess_waits(nc, maxw=1):
    """This walrus rejects instructions carrying more than one sem wait;
    move extra waits onto nops inserted just before the instruction."""
    n = 0
    for f in nc.m.functions:
        for bb in f.blocks:
            changed = False
            out = []
            for inst in bb.instructions:
                si = inst.sync_info
                waits = list(si.on_wait) if si is not None else []
                if len(waits) > maxw:
                    changed = True
                    keep = waits[-maxw:]
                    extra = waits[:-maxw]
                    for i in range(0, len(extra), maxw):
                        nop = mybir.InstNoOp(name=f"I-wsplit-{n}", ins=[], outs=[])
                        n += 1
                        nop.engine = inst.engine
                        nop.sync_info = mybir.SyncInfo(
                            on_wait=extra[i : i + maxw], on_update=[]
                        )
                        out.append(nop)
                    inst.sync_info = mybir.SyncInfo(
                        on_wait=keep, on_update=list(si.on_update)
                    )
                out.append(inst)
            if changed:
                bb.instructions = out


# ---------------------------------------------------------------- bass build
def _build_nc():
    nc = bass.Bass()
    dp = nc.declare_dram_parameter
    ins = {}
    for name, shape, dt in [
        ("xb0", [128, HW], BF16),
        ("xb1", [128, HW], BF16),
        ("xt0", [128, HW], BF16),
        ("xt1", [128, HW], BF16),
        ("xres", [HC, HW], F32),
        ("wqkv", [128, 768], BF16),  # wq0|wq1|wk0|wk1|wv0|wv1 packed
        ("bq", [128, 1], F32),
        ("mask01", [96, GRP * 384], BF16),
    ]:
        ins[name] = dp(name, shape, dt, isOutput=False)
    out_d = dp("out", [HC, HW], F32, isOutput=True)

    with tile.TileContext(nc) as tc:
        with (
            tc.tile_pool(name="consts", bufs=1) as consts,
            tc.tile_pool(name="qdpool", bufs=1) as qdpool,
            tc.tile_pool(name="persist", bufs=1) as persist,
            tc.tile_pool(name="xpool", bufs=1) as xpool,
        ):
            # ---- constants ----
            wqkv_sb = consts.tile([128, 768], BF16, name="wqkv_sb")
            nc.sync.dma_start(wqkv_sb[:], ins["wqkv"][:])
            w_tiles = {
                wname: wqkv_sb[:, 128 * i : 128 * i + 128]
                for i, wname in enumerate(
                    ("wq0", "wq1", "wk0", "wk1", "wv0", "wv1")
                )
            }
            bq_sb = consts.tile([128, 1], F32, name="bq_sb")
            nc.sync.dma_start(bq_sb[:], ins["bq"][:])
            mask01_sb = consts.tile([96, GRP * 384], BF16, name="mask01_sb")
            nc.sync.dma_start(mask01_sb[:], ins["mask01"][:])
            ones_w = consts.tile([96, 32], BF16, name="ones_w")
            nc.vector.memset(ones_w[:], 1.0)

            # ---- persistent block-diag q: [chan, row y, (head, query i)] ----
            # split into 3 row-band tiles so early energies only depend on
            # their own band's build DMAs, not the whole build sequence.
            # off-diag blocks stay zero across both phases (builds only ever
            # write the diagonal blocks), so memset exactly once, split
            # across three engines so it hides under the first x loads.
            qd_band = [
                qdpool.tile([128, 32, 384], BF16, name=f"qd_b{b}")
                for b in range(3)
            ]
            # all three memsets on gpsimd: it is idle during projection and
            # this keeps the scalar/vector queues free for x loads + copies
            for b in range(3):
                nc.gpsimd.memset(qd_band[b][:], 0.0)

            # ---- long-lived outputs of the column half ----
            # one tensor, layout [chan, x, 0:96 agg | 96:192 s] so the
            # phase-A extraction is a single copy per group
            asC = persist.tile([128, H, 192], BF16, name="asC")

            def load_chunk(n0, n1, c):
                """Stream chunk c of both x half-tensors (scalar DMA queue)."""
                t0 = xpool.tile([128, CPADF], BF16, name="xc0", tag="xc0", bufs=3)
                t1 = xpool.tile([128, CPADF], BF16, name="xc1", tag="xc1", bufs=3)
                n = CPADF if c < NCH - 1 else CHUNK
                nc.scalar.dma_start(
                    t0[:, 0:n], ins[n0][:, CHUNK * c : CHUNK * c + n]
                )
                nc.scalar.dma_start(
                    t1[:, 0:n], ins[n1][:, CHUNK * c : CHUNK * c + n]
                )
                if c == NCH - 1:
                    nc.vector.memset(t0[:, CHUNK:CPADF], 0.0)
                    nc.vector.memset(t1[:, CHUNK:CPADF], 0.0)
                return t0, t1

            def project(ppool, n0, n1, q_sb, k_sb, vT_sb, preloaded):
                """Stream x chunks; fill q_sb/k_sb [128, *] and vT_sb
                [96, 128*H]; issue qd_full build DMAs as q rows complete."""
                nc.vector.memset(k_sb[:, HW:PADF], 0.0)
                if preloaded is not None:
                    chunks = dict(preloaded)
                else:
                    chunks = {}
                    chunks[0] = load_chunk(n0, n1, 0)
                    chunks[1] = load_chunk(n0, n1, 1)
                for c in range(NCH):
                    if c + 2 < NCH:
                        chunks[c + 2] = load_chunk(n0, n1, c + 2)
                    x0, x1 = chunks.pop(c)
                    base = CHUNK * c
                    for dst, wa, wb, b_ap, ceng in (
                        (q_sb, "wq0", "wq1", bq_sb, None),
                        (k_sb, "wk0", "wk1", None, nc.scalar.copy),
                    ):
                        for n in range(CHUNK // 512):
                            lo = 512 * n
                            ps = ppool.tile(
                                [128, 512], F32, name="proj_ps", tag="proj"
                            )
                            nc.tensor.matmul(
                                ps[:], w_tiles[wa][:], x0[:, lo : lo + 512],
                                start=True, stop=False,
                            )
                            nc.tensor.matmul(
                                ps[:], w_tiles[wb][:], x1[:, lo : lo + 512],
                                start=False, stop=True,
                            )
                            d = dst[:, base + lo : base + lo + 512]
                            if b_ap is None:
                                ceng(d, ps[:])
                            else:
                                nc.vector.tensor_scalar_add(d, ps[:], b_ap[:])
                    # vT: per row y, out[i, hc] = sum_ch x[ch, 96y+i] Wv[hc, ch]
                    for y4l in range(4):
                        y4 = 4 * c + y4l
                        ps = ppool.tile([128, 512], F32, name="vt_ps", tag="proj")
                        for t in range(4):
                            yl = 384 * y4l + 96 * t
                            nc.tensor.matmul(
                                ps[:, 128 * t : 128 * t + 128],
                                x0[:, yl : yl + 128],
                                w_tiles["wv0"][:],
                                start=True, stop=False,
                            )
                            nc.tensor.matmul(
                                ps[:, 128 * t : 128 * t + 128],
                                x1[:, yl : yl + 128],
                                w_tiles["wv1"][:],
                                start=False, stop=True,
                            )
                        eng = nc.vector.tensor_copy if y4l % 2 == 0 else nc.scalar.copy
                        eng(vT_sb[:, 512 * y4 : 512 * y4 + 512], ps[0:96, :])
                    if c % 2 == 1:
                        b = c // 2  # 32 finished rows -> 4 build DMAs
                        for h in range(NHG):
                            src = q_sb[
                                32 * h : 32 * h + 32, 3072 * b : 3072 * b + 3072
                            ].rearrange("p (y i) -> p y i", i=96)
                            nc.sync.dma_start(
                                qd_band[b][
                                    32 * h : 32 * h + 32, :, 96 * h : 96 * h + 96
                                ],
                                src,
                            )

            def attention_half(pool, psum_e, psum_a, k_sb, vT_sb,
                               masked, fuse_in, prefetch_fn=None):
                """One criss-cross half over the qd bands. If fuse_in is
                False, extract agg/s into asC (column half). Otherwise
                finalize rows completely: combine with the column half,
                normalize, residual-add, store (row half)."""
                mask_v = mask01_sb.rearrange("p (a b) -> p a b", a=GRP)
                fin = {}

                def emit_energy(g):
                    e_ps = psum_e.tile([128, GRP, 512], F32, name="e_ps", tag="e")
                    for t in range(GRP):
                        y = GRP * g + t
                        nc.tensor.matmul(
                            e_ps[:, t, 0:384],
                            k_sb[:, 96 * y : 96 * y + 128],
                            qd_band[y // 32][:, y % 32, :],
                        )
                    a_sl = pool.tile(
                        [96, GRP, 384], BF16, name="a_ring", tag="a", bufs=4
                    )
                    nc.scalar.activation(a_sl, e_ps[0:96, :, 0:384], AF.Exp)
                    if masked:
                        # split the self-mask across two engines (heads 0-1 /
                        # heads 2-3) so neither becomes the per-group limiter
                        nc.vector.tensor_tensor(
                            a_sl[:, :, 0:192], a_sl[:, :, 0:192],
                            mask_v[:, :, 0:192], OP.mult,
                        )
                        nc.gpsimd.tensor_tensor(
                            a_sl[:, :, 192:384], a_sl[:, :, 192:384],
                            mask_v[:, :, 192:384], OP.mult,
                        )
                    return a_sl

                def emit_agg(g, a_sl):
                    # agg and s share one bank-sized psum tile: agg in cols
                    # 0:96, the replicated softmax-denominator sums in 96:192
                    as_ps = psum_a.tile(
                        [128, GRP, 192], F32, name="as_ps", tag="ag", bufs=2
                    )
                    for t in range(GRP):
                        y = GRP * g + t
                        for h in range(NHG):
                            nc.tensor.matmul(
                                as_ps[32 * h : 32 * h + 32, t, 0:96],
                                vT_sb[:, 128 * y + 32 * h : 128 * y + 32 * h + 32],
                                a_sl[:, t, 96 * h : 96 * h + 96],
                                tile_position=(0, 32 * h),
                            )
                    for h in range(NHG):
                        nc.tensor.matmul(
                            as_ps[32 * h : 32 * h + 32, :, 96:192],
                            ones_w[:],
                            a_sl[:, :, 96 * h : 96 * h + 96],
                            tile_position=(0, 32 * h),
                        )
                    if not fuse_in:
                        # column half: one copy lands agg AND s for this group
                        nc.vector.tensor_copy(
                            asC[:, GRP * g : GRP * g + GRP, :], as_ps[:]
                        )
                        return
                    # finalize: accumulate (row+col) agg and s into FBLK-group
                    # block tiles; normalize + residual-add once per block
                    blk, j = divmod(g, FBLK)
                    BW = FBLK * GRP * 96
                    if j == 0:
                        if blk == 0:
                            xr = pool.tile([128, BW], F32, name="xr",
                                           tag="xr", bufs=2)
                            nc.sync.dma_start(xr[:], ins["xres"][:, 0:BW])
                            fin["xr"] = xr
                        fin["agg"] = pool.tile([128, BW], F32, name="agg_blk",
                                               tag="ab", bufs=2)
                        fin["s"] = pool.tile([128, BW], F32, name="s_blk",
                                             tag="sb", bufs=2)
                        if blk + 1 < NG // FBLK:
                            nxr = pool.tile([128, BW], F32, name="xr",
                                            tag="xr", bufs=2)
                            w0 = BW * (blk + 1)
                            nc.sync.dma_start(nxr[:], ins["xres"][:, w0 : w0 + BW])
                            fin["xr_next"] = nxr
                    asC_T = asC.rearrange("p x c -> p c x")
                    aggC_T = asC_T[:, GRP * g : GRP * g + GRP, :]
                    sC_T = asC_T[:, 96 + GRP * g : 96 + GRP * g + GRP, :]
                    lo = GRP * 96 * j
                    nc.vector.tensor_tensor(
                        fin["agg"][:, lo : lo + GRP * 96].rearrange(
                            "p (a b) -> p a b", a=GRP
                        ),
                        as_ps[:, :, 0:96], aggC_T, OP.add,
                    )
                    nc.vector.tensor_tensor(
                        fin["s"][:, lo : lo + GRP * 96].rearrange(
                            "p (a b) -> p a b", a=GRP
                        ),
                        as_ps[:, :, 96:192], sC_T, OP.add,
                    )
                    def norm_store(sl0, sl1):
                        # gamma is folded into Wv on the host, so this is just
                        # normalize + residual; the store reads the xr tile.
                        # 1/s via exp(-ln s): vector.reciprocal measures ~6x
                        # slower than two scalar activation passes.
                        c = slice(sl0, sl1)
                        nc.scalar.activation(fin["s"][:, c], fin["s"][:, c],
                                             AF.Ln)
                        nc.scalar.activation(fin["s"][:, c], fin["s"][:, c],
                                             AF.Exp, scale=-1.0)
                        nc.gpsimd.tensor_mul(fin["agg"][:, c], fin["agg"][:, c],
                                             fin["s"][:, c])
                        nc.gpsimd.tensor_tensor(
                            fin["xr"][:, c], fin["agg"][:, c], fin["xr"][:, c],
                            OP.add,
                        )
                        w0 = BW * blk
                        nc.sync.dma_start(
                            out_d[:, w0 + sl0 : w0 + sl1], fin["xr"][:, c]
                        )

                    last = blk == NG // FBLK - 1
                    if last:
                        # split the final block's chain so the kernel tail
                        # isn't one long serial normalize
                        if j % 2 == 1:
                            norm_store(BW * (j - 1) // FBLK, BW * (j + 1) // FBLK)
                    elif j == FBLK - 1:
                        norm_store(0, BW)
                    if j == FBLK - 1 and "xr_next" in fin:
                        fin["xr"] = fin.pop("xr_next")

                # software pipeline: energies run two groups ahead so the
                # exp+mask chain never blocks the PE's aggregation stream
                a_ring = [emit_energy(0), emit_energy(1)]
                pre = None
                for g in range(NG):
                    if g + 2 < NG:
                        a_ring.append(emit_energy(g + 2))
                    emit_agg(g, a_ring[g])
                    if g == NG - 6 and prefetch_fn is not None:
                        # emit the next phase's first x loads here, after the
                        # attention pools' entry clocks are taken, so no
                        # attention tile inherits a dependency on them
                        pre = prefetch_fn()
                return pre

            def run_phase(n0, n1, masked, fuse_in, preloaded, prefetch):
                qk_cm = tc.tile_pool(name="qk", bufs=1)
                qk = qk_cm.__enter__()
                k_sb = qk.tile([128, PADF], BF16, name="k_sb")
                vT_sb = qk.tile([96, 128 * H], BF16, name="vT_sb")
                qp_cm = tc.tile_pool(name="qp", bufs=1)
                qp = qp_cm.__enter__()
                q_sb = qp.tile([128, HW], BF16, name="q_sb")
                with tc.tile_pool(name="proj_ps", bufs=8, space="PSUM") as ppool:
                    project(ppool, n0, n1, q_sb, k_sb, vT_sb, preloaded)
                qp_cm.__exit__(None, None, None)
                pf = None
                if prefetch is not None:
                    def pf():
                        return {
                            0: load_chunk(prefetch[0], prefetch[1], 0),
                            1: load_chunk(prefetch[0], prefetch[1], 1),
                        }
                with (
                    tc.tile_pool(name="run", bufs=1) as runp,
                    tc.tile_pool(name="ps_e", bufs=3, space="PSUM") as ps_e,
                    tc.tile_pool(name="ps_a", bufs=2, space="PSUM") as ps_a,
                ):
                    pre = attention_half(runp, ps_e, ps_a, k_sb, vT_sb,
                                         masked, fuse_in, pf)
                qk_cm.__exit__(None, None, None)
                return pre

            # phase A: column half (transposed image); phase B: row half
            pre = run_phase("xt0", "xt1", True, False, None, ("xb0", "xb1"))
            run_phase("xb0", "xb1", False, True, pre, None)

    _split_excess_waits(nc)
    return nc


# ---------------------------------------------------------------- host side
def kernel(x, Wq, bq, Wk, bk, Wv, bv, gamma):
    x = np.asarray(x, np.float32)
    Wq, bq = np.asarray(Wq, np.float32), np.asarray(bq, np.float32)
    Wk, bk = np.asarray(Wk, np.float32), np.asarray(bk, np.float32)
    Wv, bv = np.asarray(Wv, np.float32), np.asarray(bv, np.float32)
    gamma = np.asarray(gamma, np.float32)
    bf16 = ml_dtypes.bfloat16

    if "nc" not in _cached:
        _cached["nc"] = _build_nc()
    nc = _cached["nc"]

    eye = np.eye(96, dtype=bool)
    mask1 = np.where(eye, np.float32(0.0), np.float32(1.0))  # [z, y]
    mask384 = np.tile(mask1, (1, NHG))  # [96, 384] blocks (h, y)
    mask01 = np.tile(mask384, (1, GRP)).astype(bf16)

    in_maps = []
    for core in range(8):
        b, hg = core // 2, core % 2
        xb = np.ascontiguousarray(x[b].reshape(C, HW))
        xt = np.ascontiguousarray(
            x[b].reshape(C, H, W).transpose(0, 2, 1).reshape(C, HW)
        )
        r = slice(128 * hg, 128 * hg + 128)
        m = {
            "xb0": xb[:128].astype(bf16),
            "xb1": xb[128:].astype(bf16),
            "xt0": xt[:128].astype(bf16),
            "xt1": xt[128:].astype(bf16),
            "xres": xb[r] + gamma[0] * bv[r][:, None],
            "wqkv": np.ascontiguousarray(
                np.concatenate(
                    [
                        Wq[r, 0:128].T, Wq[r, 128:256].T,
                        Wk[r, 0:128].T, Wk[r, 128:256].T,
                        gamma[0] * Wv[r, 0:128].T, gamma[0] * Wv[r, 128:256].T,
                    ],
                    axis=1,
                )
            ).astype(bf16),
            "bq": np.ascontiguousarray(bq[r]).reshape(128, 1),
            "mask01": mask01,
        }
        in_maps.append(m)

    import os

    trace = os.environ.get("BASS_KERNEL_TRACE", "0") == "1"
    res = run_bass_kernel_spmd(
        nc, in_maps, core_ids=list(range(8)), trace=trace
    )
    _cached["last_res"] = res
    out = np.empty((B, C, H, W), np.float32)
    for core in range(8):
        b, hg = core // 2, core % 2
        out[b, 128 * hg : 128 * hg + 128] = res.results[core]["out"].reshape(
            128, H, W
        )
    return out


# revision 38
# speedup vs baseline: 1.0080x; 1.0080x over previous
"""Criss-Cross Attention (B=4, C=256, H=W=96, 8 heads) on 8 trn2 NeuronCores.

Sharding: core = (batch b, head-group hg of 4 heads); 8 cores = 4 b x 2 hg.
Each core computes q/k/v projections for its 128 output channels over the
full image, then row- and column-attention for its 4 heads, then
gamma*out + x for its channel slice. Host splits inputs / concatenates
outputs; no collectives.

Key perf structure: a persistent block-diagonal q tensor (qd_full) is
built with a few large DMAs per phase (DMA *issue* costs ~640ns on the
issuing queue regardless of size, so instruction count is what matters);
x is streamed in chunks on the scalar DMA queue; the column-half
self-mask is a post-exp 0/1 multiply on gpsimd; energies run two groups
ahead of aggregation so the PE never starves.

Self-contained: includes the TileContext drain-wait splitting workaround.
"""

import numpy as np
import ml_dtypes

import concourse.bass as bass
import concourse.mybir as mybir
import concourse.tile as tile
from concourse.vector_clock import ScopedClock
from concourse.bass_utils import run_bass_kernel_spmd

BF16 = mybir.dt.bfloat16
F32 = mybir.dt.float32
AF = mybir.ActivationFunctionType
OP = mybir.AluOpType

B, C, H, W = 4, 256, 96, 96
HW = H * W  # 9216
HC = 128  # output channels per core (4 heads x 32)
NHG = 4  # heads per core
HD = 32
GRP = 2  # rows per pipeline group
NG = H // GRP  # 48 groups
PADF = HW + 32  # k free-dim pad so [.., 96y : 96y+128] reads stay in-bounds
CHUNK = 1536  # pixels per x streaming chunk (16 rows)
NCH = HW // CHUNK  # 6
CPADF = CHUNK + 32  # chunk pad for vT's 128-wide row windows
FBLK = 6  # finalize groups per xres-load / out-store block

_cached = {}


# ---------------------------------------------------------------- tile patch
def _patched_drain_and_barrier(self, tick_clock, wait_clock):
    # walrus here rejects >1 sem wait on the final drain; spread the
    # global-clock waits across single-wait nops.
    drain_inst = self.nc.sync.drain()
    wait_clock.add_sem_waits(
        drain_inst.ins, ScopedClock({None: tick_clock.global_clock})
    )
    si = drain_inst.ins.sync_info
    waits = list(si.on_wait)
    if len(waits) > 1:
        drain_inst.ins.sync_info = mybir.SyncInfo(
            on_wait=waits[:1], on_update=list(si.on_update)
        )
        for i in range(1, len(waits)):
            nop = self.nc.sync.nop(nofuse=True)
            nop.ins.sync_info = mybir.SyncInfo(on_wait=waits[i : i + 1], on_update=[])
    self.nc.all_engine_barrier()
    assert self.sems is not None
    popped = self.nc._tile_sem_poison_stack.pop()
    assert popped is self._sem_poison
    self.nc.clear_and_free_semaphores(list(self.sems.allocated().values()))
    self.nc.all_engine_barrier()


tile.TileContext._drain_and_barrier = _patched_drain_and_barrier


def _split_excess_waits(nc, maxw=1):
    """This walrus rejects instructions carrying more than one sem wait;
    move extra waits onto nops inserted just before the instruction."""
    n = 0
    for f in nc.m.functions:
        for bb in f.blocks:
            changed = False
            out = []
            for inst in bb.instructions:
                si = inst.sync_info
                waits = list(si.on_wait) if si is not None else []
                if len(waits) > maxw:
                    changed = True
                    keep = waits[-maxw:]
                    extra = waits[:-maxw]
                    for i in range(0, len(extra), maxw):
                        nop = mybir.InstNoOp(name=f"I-wsplit-{n}", ins=[], outs=[])
                        n += 1
                        nop.engine = inst.engine
                        nop.sync_info = mybir.SyncInfo(
                            on_wait=extra[i : i + maxw], on_update=[]
                        )
                        out.append(nop)
                    inst.sync_info = mybir.SyncInfo(
                        on_wait=keep, on_update=list(si.on_update)
                    )
                out.append(inst)
            if changed:
                bb.instructions = out


# ---------------------------------------------------------------- bass build
def _build_nc():
    nc = bass.Bass()
    dp = nc.declare_dram_parameter
    ins = {}
    for name, shape, dt in [
        ("xb0", [128, HW], BF16),
        ("xb1", [128, HW], BF16),
        ("xt0", [128, HW], BF16),
        ("xt1", [128, HW], BF16),
        ("xres", [HC, HW], F32),
        ("wqkv", [128, 768], BF16),  # wq0|wq1|wk0|wk1|wv0|wv1 packed
        ("bq", [128, 1], F32),
        ("mask01", [96, GRP * 384], BF16),
    ]:
        ins[name] = dp(name, shape, dt, isOutput=False)
    out_d = dp("out", [HC, HW], F32, isOutput=True)

    with tile.TileContext(nc) as tc:
        with (
            tc.tile_pool(name="consts", bufs=1) as consts,
            tc.tile_pool(name="qdpool", bufs=1) as qdpool,
            tc.tile_pool(name="persist", bufs=1) as persist,
            tc.tile_pool(name="xpool", bufs=1) as xpool,
        ):
            # ---- constants ----
            wqkv_sb = consts.tile([128, 768], BF16, name="wqkv_sb")
            nc.sync.dma_start(wqkv_sb[:], ins["wqkv"][:])
            w_tiles = {
                wname: wqkv_sb[:, 128 * i : 128 * i + 128]
                for i, wname in enumerate(
                    ("wq0", "wq1", "wk0", "wk1", "wv0", "wv1")
                )
            }
            bq_sb = consts.tile([128, 1], F32, name="bq_sb")
            nc.sync.dma_start(bq_sb[:], ins["bq"][:])
            mask01_sb = consts.tile([96, GRP * 384], BF16, name="mask01_sb")
            nc.sync.dma_start(mask01_sb[:], ins["mask01"][:])
            ones_w = consts.tile([96, 32], BF16, name="ones_w")
            nc.vector.memset(ones_w[:], 1.0)

            # ---- persistent block-diag q: [chan, row y, (head, query i)] ----
            # split into 3 row-band tiles so early energies only depend on
            # their own band's build DMAs, not the whole build sequence.
            # off-diag blocks stay zero across both phases (builds only ever
            # write the diagonal blocks), so memset exactly once, split
            # across three engines so it hides under the first x loads.
            qd_band = [
                qdpool.tile([128, 32, 384], BF16, name=f"qd_b{b}")
                for b in range(3)
            ]
            # all three memsets on gpsimd: it is idle during projection and
            # this keeps the scalar/vector queues free for x loads + copies
            for b in range(3):
                nc.gpsimd.memset(qd_band[b][:], 0.0)

            # ---- long-lived outputs of the column half ----
            # one tensor, layout [chan, x, 0:96 agg | 96:192 s] so the
            # phase-A extraction is a single copy per group
            asC = persist.tile([128, H, 192], BF16, name="asC")

            def load_chunk(n0, n1, c):
                """Stream chunk c of both x half-tensors (scalar DMA queue)."""
                t0 = xpool.tile([128, CPADF], BF16, name="xc0", tag="xc0", bufs=3)
                t1 = xpool.tile([128, CPADF], BF16, name="xc1", tag="xc1", bufs=3)
                n = CPADF if c < NCH - 1 else CHUNK
                nc.scalar.dma_start(
                    t0[:, 0:n], ins[n0][:, CHUNK * c : CHUNK * c + n]
                )
                nc.scalar.dma_start(
                    t1[:, 0:n], ins[n1][:, CHUNK * c : CHUNK * c + n]
                )
                if c == NCH - 1:
                    nc.vector.memset(t0[:, CHUNK:CPADF], 0.0)
                    nc.vector.memset(t1[:, CHUNK:CPADF], 0.0)
                return t0, t1

            def project(ppool, n0, n1, q_sb, k_sb, vT_sb, preloaded):
                """Stream x chunks; fill q_sb/k_sb [128, *] and vT_sb
                [96, 128*H]; issue qd_full build DMAs as q rows complete."""
                nc.vector.memset(k_sb[:, HW:PADF], 0.0)
                if preloaded is not None:
                    chunks = dict(preloaded)
                else:
                    chunks = {}
                    chunks[0] = load_chunk(n0, n1, 0)
                    chunks[1] = load_chunk(n0, n1, 1)
                for c in range(NCH):
                    if c + 2 < NCH:
                        chunks[c + 2] = load_chunk(n0, n1, c + 2)
                    x0, x1 = chunks.pop(c)
                    base = CHUNK * c
                    for dst, wa, wb, b_ap, ceng in (
                        (q_sb, "wq0", "wq1", bq_sb, None),
                        (k_sb, "wk0", "wk1", None, nc.scalar.copy),
                    ):
                        for n in range(CHUNK // 512):
                            lo = 512 * n
                            ps = ppool.tile(
                                [128, 512], F32, name="proj_ps", tag="proj"
                            )
                            nc.tensor.matmul(
                                ps[:], w_tiles[wa][:], x0[:, lo : lo + 512],
                                start=True, stop=False,
                            )
                            nc.tensor.matmul(
                                ps[:], w_tiles[wb][:], x1[:, lo : lo + 512],
                                start=False, stop=True,
                            )
                            d = dst[:, base + lo : base + lo + 512]
                            if b_ap is None:
                                ceng(d, ps[:])
                            else:
                                nc.vector.tensor_scalar_add(d, ps[:], b_ap[:])
                    # vT: per row y, out[i, hc] = sum_ch x[ch, 96y+i] Wv[hc, ch]
                    for y4l in range(4):
                        y4 = 4 * c + y4l
                        ps = ppool.tile([128, 512], F32, name="vt_ps", tag="proj")
                        for t in range(4):
                            yl = 384 * y4l + 96 * t
                            nc.tensor.matmul(
                                ps[:, 128 * t : 128 * t + 128],
                                x0[:, yl : yl + 128],
                                w_tiles["wv0"][:],
                                start=True, stop=False,
                            )
                            nc.tensor.matmul(
                                ps[:, 128 * t : 128 * t + 128],
                                x1[:, yl : yl + 128],
                                w_tiles["wv1"][:],
                                start=False, stop=True,
                            )
                        eng = nc.vector.tensor_copy if y4l % 2 == 0 else nc.scalar.copy
                        eng(vT_sb[:, 512 * y4 : 512 * y4 + 512], ps[0:96, :])
                    if c % 2 == 1:
                        b = c // 2  # 32 finished rows -> 4 build DMAs
                        for h in range(NHG):
                            src = q_sb[
                                32 * h : 32 * h + 32, 3072 * b : 3072 * b + 3072
                            ].rearrange("p (y i) -> p y i", i=96)
                            nc.sync.dma_start(
                                qd_band[b][
                                    32 * h : 32 * h + 32, :, 96 * h : 96 * h + 96
                                ],
                                src,
                            )

            def attention_half(pool, psum_e, psum_a, k_sb, vT_sb,
                               masked, fuse_in, prefetch_fn=None):
                """One criss-cross half over the qd bands. If fuse_in is
                False, extract agg/s into asC (column half). Otherwise
                finalize rows completely: combine with the column half,
                normalize, residual-add, store (row half)."""
                mask_v = mask01_sb.rearrange("p (a b) -> p a b", a=GRP)
                fin = {}

                def emit_energy(g):
                    e_ps = psum_e.tile([128, GRP, 512], F32, name="e_ps", tag="e")
                    for t in range(GRP):
                        y = GRP * g + t
                        nc.tensor.matmul(
                            e_ps[:, t, 0:384],
                            k_sb[:, 96 * y : 96 * y + 128],
                            qd_band[y // 32][:, y % 32, :],
                        )
                    a_sl = pool.tile(
                        [96, GRP, 384], BF16, name="a_ring", tag="a", bufs=4
                    )
                    nc.scalar.activation(a_sl, e_ps[0:96, :, 0:384], AF.Exp)
                    if masked:
                        # split the self-mask across two engines (heads 0-1 /
                        # heads 2-3) so neither becomes the per-group limiter
                        nc.vector.tensor_tensor(
                            a_sl[:, :, 0:192], a_sl[:, :, 0:192],
                            mask_v[:, :, 0:192], OP.mult,
                        )
                        nc.gpsimd.tensor_tensor(
                            a_sl[:, :, 192:384], a_sl[:, :, 192:384],
                            mask_v[:, :, 192:384], OP.mult,
                        )
                    return a_sl

                def emit_agg(g, a_sl):
                    # agg and s share one bank-sized psum tile: agg in cols
                    # 0:96, the replicated softmax-denominator sums in 96:192
                    as_ps = psum_a.tile(
                        [128, GRP, 192], F32, name="as_ps", tag="ag", bufs=2
                    )
                    for t in range(GRP):
                        y = GRP * g + t
                        for h in range(NHG):
                            nc.tensor.matmul(
                                as_ps[32 * h : 32 * h + 32, t, 0:96],
                                vT_sb[:, 128 * y + 32 * h : 128 * y + 32 * h + 32],
                                a_sl[:, t, 96 * h : 96 * h + 96],
                                tile_position=(0, 32 * h),
                            )
                    for h in range(NHG):
                        nc.tensor.matmul(
                            as_ps[32 * h : 32 * h + 32, :, 96:192],
                            ones_w[:],
                            a_sl[:, :, 96 * h : 96 * h + 96],
                            tile_position=(0, 32 * h),
                        )
                    if not fuse_in:
                        # column half: one copy lands agg AND s for this group
                        nc.vector.tensor_copy(
                            asC[:, GRP * g : GRP * g + GRP, :], as_ps[:]
                        )
                        return
                    # finalize: accumulate (row+col) agg and s into FBLK-group
                    # block tiles; normalize + residual-add once per block
                    blk, j = divmod(g, FBLK)
                    BW = FBLK * GRP * 96
                    if j == 0:
                        if blk == 0:
                            xr = pool.tile([128, BW], F32, name="xr",
                                           tag="xr", bufs=2)
                            nc.sync.dma_start(xr[:], ins["xres"][:, 0:BW])
                            fin["xr"] = xr
                        fin["agg"] = pool.tile([128, BW], F32, name="agg_blk",
                                               tag="ab", bufs=2)
                        fin["s"] = pool.tile([128, BW], F32, name="s_blk",
                                             tag="sb", bufs=2)
                        if blk + 1 < NG // FBLK:
                            nxr = pool.tile([128, BW], F32, name="xr",
                                            tag="xr", bufs=2)
                            w0 = BW * (blk + 1)
                            nc.sync.dma_start(nxr[:], ins["xres"][:, w0 : w0 + BW])
                            fin["xr_next"] = nxr
                    asC_T = asC.rearrange("p x c -> p c x")
                    aggC_T = asC_T[:, GRP * g : GRP * g + GRP, :]
                    sC_T = asC_T[:, 96 + GRP * g : 96 + GRP * g + GRP, :]
                    lo = GRP * 96 * j
                    nc.vector.tensor_tensor(
                        fin["agg"][:, lo : lo + GRP * 96].rearrange(
                            "p (a b) -> p a b", a=GRP
                        ),
                        as_ps[:, :, 0:96], aggC_T, OP.add,
                    )
                    nc.vector.tensor_tensor(
                        fin["s"][:, lo : lo + GRP * 96].rearrange(
                            "p (a b) -> p a b", a=GRP
                        ),
                        as_ps[:, :, 96:192], sC_T, OP.add,
                    )
                    def norm_store(sl0, sl1):
                        # gamma is folded into Wv on the host, so this is just
                        # normalize + residual; the store reads the xr tile.
                        # 1/s via exp(-ln s): vector.reciprocal measures ~6x
                        # slower than two scalar activation passes.
                        c = slice(sl0, sl1)
                        nc.scalar.activation(fin["s"][:, c], fin["s"][:, c],
                                             AF.Ln)
                        nc.scalar.activation(fin["s"][:, c], fin["s"][:, c],
                                             AF.Exp, scale=-1.0)
                        nc.gpsimd.tensor_mul(fin["agg"][:, c], fin["agg"][:, c],
                                             fin["s"][:, c])
                        nc.gpsimd.tensor_tensor(
                            fin["xr"][:, c], fin["agg"][:, c], fin["xr"][:, c],
                            OP.add,
                        )
                        w0 = BW * blk
                        nc.sync.dma_start(
                            out_d[:, w0 + sl0 : w0 + sl1], fin["xr"][:, c]
                        )

                    last = blk == NG // FBLK - 1
                    if last:
                        # split the final block's chain so the kernel tail
                        # isn't one long serial normalize
                        if j % 2 == 1:
                            norm_store(BW * (j - 1) // FBLK, BW * (j + 1) // FBLK)
                    elif j == FBLK - 1:
                        norm_store(0, BW)
                    if j == FBLK - 1 and "xr_next" in fin:
                        fin["xr"] = fin.pop("xr_next")

                # software pipeline: energies run two groups ahead so the
                # exp+mask chain never blocks the PE's aggregation stream
                a_ring = [emit_energy(0), emit_energy(1)]
                pre = None
                for g in range(NG):
                    if g + 2 < NG:
                        a_ring.append(emit_energy(g + 2))
                    emit_agg(g, a_ring[g])
                    if g == 2 and prefetch_fn is not None:
                        # emit the next phase's first x loads here, after the
                        # attention pools' entry clocks are taken, so no
                        # attention tile inherits a dependency on them
                        pre = prefetch_fn()
                return pre

            def run_phase(n0, n1, masked, fuse_in, preloaded, prefetch):
                qk_cm = tc.tile_pool(name="qk", bufs=1)
                qk = qk_cm.__enter__()
                k_sb = qk.tile([128, PADF], BF16, name="k_sb")
                vT_sb = qk.tile([96, 128 * H], BF16, name="vT_sb")
                qp_cm = tc.tile_pool(name="qp", bufs=1)
                qp = qp_cm.__enter__()
                q_sb = qp.tile([128, HW], BF16, name="q_sb")
                with tc.tile_pool(name="proj_ps", bufs=8, space="PSUM") as ppool:
                    project(ppool, n0, n1, q_sb, k_sb, vT_sb, preloaded)
                qp_cm.__exit__(None, None, None)
                pf = None
                if prefetch is not None:
                    def pf():
                        return {
                            0: load_chunk(prefetch[0], prefetch[1], 0),
                            1: load_chunk(prefetch[0], prefetch[1], 1),
                        }
                with (
                    tc.tile_pool(name="run", bufs=1) as runp,
                    tc.tile_pool(name="ps_e", bufs=3, space="PSUM") as ps_e,
                    tc.tile_pool(name="ps_a", bufs=2, space="PSUM") as ps_a,
                ):
                    pre = attention_half(runp, ps_e, ps_a, k_sb, vT_sb,
                                         masked, fuse_in, pf)
                qk_cm.__exit__(None, None, None)
                return pre

            # phase A: column half (transposed image); phase B: row half
            pre = run_phase("xt0", "xt1", True, False, None, ("xb0", "xb1"))
            run_phase("xb0", "xb1", False, True, pre, None)

    _split_excess_waits(nc)
    return nc


# ---------------------------------------------------------------- host side
def kernel(x, Wq, bq, Wk, bk, Wv, bv, gamma):
    x = np.asarray(x, np.float32)
    Wq, bq = np.asarray(Wq, np.float32), np.asarray(bq, np.float32)
    Wk, bk = np.asarray(Wk, np.float32), np.asarray(bk, np.float32)
    Wv, bv = np.asarray(Wv, np.float32), np.asarray(bv, np.float32)
    gamma = np.asarray(gamma, np.float32)
    bf16 = ml_dtypes.bfloat16

    if "nc" not in _cached:
        _cached["nc"] = _build_nc()
    nc = _cached["nc"]

    eye = np.eye(96, dtype=bool)
    mask1 = np.where(eye, np.float32(0.0), np.float32(1.0))  # [z, y]
    mask384 = np.tile(mask1, (1, NHG))  # [96, 384] blocks (h, y)
    mask01 = np.tile(mask384, (1, GRP)).astype(bf16)

    in_maps = []
    for core in range(8):
        b, hg = core // 2, core % 2
        xb = np.ascontiguousarray(x[b].reshape(C, HW))
        xt = np.ascontiguousarray(
            x[b].reshape(C, H, W).transpose(0, 2, 1).reshape(C, HW)
        )
        r = slice(128 * hg, 128 * hg + 128)
        m = {
            "xb0": xb[:128].astype(bf16),
            "xb1": xb[128:].astype(bf16),
            "xt0": xt[:128].astype(bf16),
            "xt1": xt[128:].astype(bf16),
            "xres": xb[r] + gamma[0] * bv[r][:, None],
            "wqkv": np.ascontiguousarray(
                np.concatenate(
                    [
                        Wq[r, 0:128].T, Wq[r, 128:256].T,
                        Wk[r, 0:128].T, Wk[r, 128:256].T,
                        gamma[0] * Wv[r, 0:128].T, gamma[0] * Wv[r, 128:256].T,
                    ],
                    axis=1,
                )
            ).astype(bf16),
            "bq": np.ascontiguousarray(bq[r]).reshape(128, 1),
            "mask01": mask01,
        }
        in_maps.append(m)

    import os

    trace = os.environ.get("BASS_KERNEL_TRACE", "0") == "1"
    res = run_bass_kernel_spmd(
        nc, in_maps, core_ids=list(range(8)), trace=trace
    )
    _cached["last_res"] = res
    out = np.empty((B, C, H, W), np.float32)
    for core in range(8):
        b, hg = core // 2, core % 2
        out[b, 128 * hg : 128 * hg + 128] = res.results[core]["out"].reshape(
            128, H, W
        )
    return out
